# revision 17
# baseline (speedup 1.0000x reference)
"""Fused multi-head attention block on 8 TRN2 NeuronCores.

reference: qkv = x@Wqkv+b; q,k rmsnorm'd per head; softmax(q k^T/sqrt(hd)) v; proj.
Shapes: x [2,2048,1024], H=16 heads, hd=64.

Distribution (no collectives): 8 cores = 2 batches x 4 head-groups (4 heads each).
Core c: batch b=c//4, heads 4g..4g+3 (g=c%4). Each core computes the partial
projection output (proj_w row-sharded over its heads) for its batch; the host
sums the 4 partials per batch and adds proj_b.

Per-core pipeline (bf16 matmul operands, f32 PSUM accumulation):
  A) qkv GEMM with OUTPUT channel-major for q,k (wqkv stationary, x^T moving)
     so q^T/k^T need no PE transposes; v computed token-major (x^T stationary)
     straight into the AV stationary layout. x arrives in 16 fine (kt,half)
     DMA pieces; tb0 runs its 4 qk chains + 2 v chains interleaved per-kt at
     DMA pace so the PE starts ~8us in and never idles long enough to lose
     the HAM p-state. rmsnorm per head: sq = qsb*qsb on DVE (w folded into
     wqkv on host); per-head sum-of-squares via a block-diagonal 1/(64 w^2)
     matmul that REPLICATES the stat across the head's 64 partitions; sqrt on
     ACT; reciprocal_approx_fast on DVE; one bf16 multiply applies rstd.
     k channels are normalized first; the two leftover norm chains at the
     A-tail are q channels needed only by the last query block.
  B) attention per (q-512-chunk, head-pair): the two heads' K=64 score matmuls
     run in separate PE row-groups (partitions 0:64 / 64:128). exp(s/8)
     alternates per k-tile between ACT (Exp activation) and DVE (Schraudolph
     bf16 bit-trick: s*a+b -> int32, read high half-words as bf16). AV
     accumulates per head with stationary [v|ones]/[ones|v] giving out^T and
     the softmax denominator in one pass; po is split per head (1 PSUM bank
     each) so the normalize epilogue (recip + bf16 multiply) pipelines and
     frees po before the next group's first AV needs it. The partial
     projection for the PREVIOUS q-chunk is interleaved into each group (one
     tile mid-group, one at the boundary) so the PE stays dense across group
     boundaries; only the last q-chunk's 4 proj tiles run as a tail.
"""

from collections import deque
from contextlib import ExitStack

import ml_dtypes
import numpy as np

import concourse.bass as bass
import concourse.mybir as mybir
import concourse.tile as tile
from concourse import bacc
from concourse.bass_utils import run_bass_kernel_spmd

B, N, C = 2, 2048, 1024
H, HD = 16, 64
HPC = 4                 # heads per core
NT = N // 128           # 16 k-token tiles
KT8 = C // 128          # 8 contraction tiles for the qkv GEMM
QK = 2 * HPC * HD       # 512 qk channels per core
V = HPC * HD            # 256 v channels per core
EPS = 1e-6
F32 = mybir.dt.float32
BF16 = mybir.dt.bfloat16
I16 = mybir.dt.int16
AF = mybir.ActivationFunctionType
MUL = mybir.AluOpType.mult
ADD = mybir.AluOpType.add

LOG2E = 1.4426950408889634
# exp(s/8) ~= bf16_frombits(int16(s*A16 + B16)): schraudolph with the /8
# softmax scale folded in; B16 centers the sawtooth error (C ~= 5.5/128).
A16 = 128.0 * LOG2E / 8.0
B16 = 16251.0

# which kt tiles' exp goes to DVE (schraudolph) instead of ACT; alternation
# keeps the score ring advancing. kts 0,1 of a pre-emitted group are
# schraudolph'd on DVE BEFORE the boundary (see run_group), and kts 2,14,15
# stay on ACT so the DVE epilogue (rec+mul) never blocks the exp chain.
DVE_KTS = frozenset((3, 5, 7, 9, 11, 13))

CBO = (2, 3, 0, 1)      # k channel-blocks first: A-tail leftovers are q-only

QUAKE = 0x5F3759DF      # rsqrt seed magic (gpsimd leftover-norm path)


def build_nc():
    nc = bacc.Bacc("TRN2", target_bir_lowering=False, debug=False)

    x_ext = nc.declare_dram_parameter("x", [C, N], BF16, isOutput=False)
    wqkv_ext = nc.declare_dram_parameter("wqkv", [C, QK + V], BF16, isOutput=False)
    bqk_ext = nc.declare_dram_parameter("bqk", [128, 4], F32, isOutput=False)
    bv_ext = nc.declare_dram_parameter("bv", [V], F32, isOutput=False)
    inc_ext = nc.declare_dram_parameter("inc", [128, 2, 128], BF16, isOutput=False)
    wproj_ext = nc.declare_dram_parameter("wproj", [V // 2, 2, C], BF16, isOutput=False)
    # bf16 partials: halves the 8 MB output drain; host sums in f32
    out_ext = nc.declare_dram_parameter("out", [N, C], BF16, isOutput=True)

    with tile.TileContext(nc) as tc, ExitStack() as ctx:
        singles = ctx.enter_context(tc.tile_pool(name="singles", bufs=1))

        wqkv_sb = singles.tile([128, KT8, QK + V], BF16, tag="wqkv")
        xt_all = singles.tile([128, KT8, N], BF16, tag="xt_all")
        # x in 16 fine pieces: h0 (tokens 0:1024) kt-major on the scalar
        # queue so (w[kt], x[kt,h0]) pairs land together ~1.25us apart and
        # tb0/tb1 can consume them at DMA pace; w + h1 on the sync queue.
        for kt in range(KT8):
            nc.scalar.dma_start(
                out=xt_all[:, kt, 0:1024],
                in_=x_ext[kt * 128:(kt + 1) * 128, 0:1024])
        for kt in range(KT8):
            nc.sync.dma_start(
                out=wqkv_sb[:, kt, :], in_=wqkv_ext[kt * 128:(kt + 1) * 128, :]
            )
        bqk_sb = singles.tile([128, 4], F32, tag="bqk")
        nc.sync.dma_start(out=bqk_sb, in_=bqk_ext[:, :])
        bv_sb = singles.tile([128, V], F32, tag="bv")
        nc.sync.dma_start(out=bv_sb, in_=bv_ext[:].partition_broadcast(128))
        inc_sb = singles.tile([128, 2, 128], BF16, tag="inc")
        nc.sync.dma_start(out=inc_sb, in_=inc_ext[:, :, :])
        for kt in range(KT8):
            nc.sync.dma_start(
                out=xt_all[:, kt, 1024:2048],
                in_=x_ext[kt * 128:(kt + 1) * 128, 1024:2048])
        wproj_sb = singles.tile([128, 2, C], BF16, tag="wproj")
        nc.sync.dma_start(out=wproj_sb, in_=wproj_ext[:, :, :])
        eps_sb = singles.tile([128, 1], F32, tag="eps")
        nc.vector.memset(eps_sb, EPS)
        junk = singles.tile([128, 1], F32, tag="junk")

        # persistent activations (all channel-major)
        qn = singles.tile([128, 2, N], BF16, tag="qn")
        kn = singles.tile([128, 2, N], BF16, tag="kn")
        # AV stationary blocks: even local head: [v | ones]; odd: [ones | v]
        # memset on gpsimd: it is otherwise idle and finishes by ~14us,
        # before the first v_finish writes (~18us)
        vaug = singles.tile([128, NT, HPC, 128], BF16, tag="vaug")
        nc.gpsimd.memset(vaug, 1.0)
        # normalized attention output; block hb packs heads 2hb,2hb+1
        aT = singles.tile([128, 2, N], BF16, tag="aT")

        # phase-B SBUF pools allocated BEFORE phase A's so that B's first
        # tiles never alias just-freed A tiles (the aliasing serialized the
        # first exp behind the A-tail norm chains)
        ptpool = ctx.enter_context(tc.tile_pool(name="pt", bufs=4))
        rpool = ctx.enter_context(tc.tile_pool(name="rec", bufs=4))
        outpool = ctx.enter_context(tc.tile_pool(name="outsb", bufs=6))

        # phase-A SBUF pools stay open through phase B: the two leftover
        # norm chains (tb3's q channels, needed only by the last q-chunk)
        # are finished inside phase B from these tiles
        qsbp = ctx.enter_context(tc.tile_pool(name="qsb", bufs=3))
        sqp = ctx.enter_context(tc.tile_pool(name="sqp", bufs=2))
        smp = ctx.enter_context(tc.tile_pool(name="smp", bufs=2))
        rstdp = ctx.enter_context(tc.tile_pool(name="rstd", bufs=2))

        # ---------------- phase A: qkv + rmsnorm, no transposes --------------
        with ExitStack() as actx:
            pqk = actx.enter_context(tc.tile_pool(name="pqk", bufs=4, space="PSUM"))
            pv = actx.enter_context(tc.tile_pool(name="pv", bufs=2, space="PSUM"))
            pssq = actx.enter_context(tc.tile_pool(name="pssq", bufs=2, space="PSUM"))

            # short PE warmup during the DMA lead-in (first data ~7.9us)
            wjunk = qsbp.tile([128, 512], BF16, tag="qsb")
            nc.vector.memset(wjunk, 0.0)
            p_warm = pssq.tile([128, 512], F32, tag="pssq")
            for _ in range(6):
                nc.tensor.matmul(p_warm[:, 0:128], wjunk[:, 0:128],
                                 wjunk[:, 0:128], start=True, stop=True)

            pend = deque()  # (qsb, sq, cb, tb) awaiting ssq matmul + rstd chain

            def evict_block(p_qk, cb, tb):
                """f32 PSUM -> bf16 qsb (+bias) on ACT; sq = qsb^2 on DVE."""
                qsb = qsbp.tile([128, 512], BF16, tag="qsb")
                nc.scalar.activation(qsb, p_qk, AF.Identity,
                                     bias=bqk_sb[:, cb:cb + 1], scale=1.0)
                sq = sqp.tile([128, 512], BF16, tag="sq")
                nc.vector.tensor_mul(sq, qsb, qsb)
                pend.append((qsb, sq, cb, tb))

            def finish_norm():
                qsb, sq, cb, tb = pend.popleft()
                tsl = slice(tb * 512, (tb + 1) * 512)
                p_ssq = pssq.tile([128, 512], F32, tag="pssq")
                nc.tensor.matmul(p_ssq, inc_sb[:, cb // 2, :], sq,
                                 start=True, stop=True)
                sm = smp.tile([128, 512], F32, tag="sm")
                nc.scalar.activation(sm, p_ssq, AF.Sqrt,
                                     bias=eps_sb[:, 0:1], scale=1.0)
                rstd = rstdp.tile([128, 512], F32, tag="rstd")
                nc.vector.reciprocal_approx_fast(rstd, sm)
                dst = qn if cb < 2 else kn
                nc.vector.tensor_tensor(dst[:, cb % 2, tsl], qsb, rstd, op=MUL)

            def v_finish(t, p_v):
                pv3 = p_v.rearrange("p (h d) -> p h d", d=HD)
                bv3 = bv_sb.rearrange("p (h d) -> p h d", d=HD)
                nc.vector.tensor_add(vaug[:, t, 0::2, 0:HD], pv3[:, 0::2, :],
                                     bv3[:, 0::2, :])
                nc.vector.tensor_add(vaug[:, t, 1::2, HD:128], pv3[:, 1::2, :],
                                     bv3[:, 1::2, :])

            # --- tb0: 4 qk chains + 2 v chains interleaved per-kt, paced by
            # the (w[kt], x[kt,h0]) DMA arrivals; junk matmuls pad the seams
            tsl0 = slice(0, 512)
            chains = {cb: pqk.tile([128, 512], F32, tag="pqk", name=f"chain{cb}")
                      for cb in CBO}
            pv01 = [pv.tile([128, V], F32, tag="pv", name=f"pv0{t}")
                    for t in range(2)]
            for kt in range(KT8):
                for cb in CBO:
                    nc.tensor.matmul(
                        chains[cb], wqkv_sb[:, kt, cb * 128:(cb + 1) * 128],
                        xt_all[:, kt, tsl0],
                        start=(kt == 0), stop=(kt == KT8 - 1),
                    )
                for t in range(2):
                    ts = slice(t * 128, (t + 1) * 128)
                    nc.tensor.matmul(
                        pv01[t], xt_all[:, kt, ts], wqkv_sb[:, kt, QK:QK + V],
                        start=(kt == 0), stop=(kt == KT8 - 1),
                    )
                if kt < KT8 - 1:
                    for _ in range(2):
                        nc.tensor.matmul(p_warm[:, 0:128], wjunk[:, 0:128],
                                         wjunk[:, 0:128], start=True, stop=True)
            for cb in CBO:
                evict_block(chains[cb], cb, 0)
            for t in range(2):
                v_finish(t, pv01[t])
            # bridge: v t2,t3 chains + first two norm finishes cover the ACT
            # eviction latency before tb1's chains reuse the pqk ring
            for t in (2, 3):
                p_v = pv.tile([128, V], F32, tag="pv")
                ts = slice(t * 128, (t + 1) * 128)
                for kt in range(KT8):
                    nc.tensor.matmul(
                        p_v, xt_all[:, kt, ts], wqkv_sb[:, kt, QK:QK + V],
                        start=(kt == 0), stop=(kt == KT8 - 1),
                    )
                finish_norm()
                v_finish(t, p_v)

            # --- tb1..tb3: sequential blocks, one v tile + one norm finish
            # per block (pend stays 2 deep; leftovers are tb3's q channels)
            for tb in range(1, 4):
                tsl = slice(tb * 512, (tb + 1) * 512)
                for i, cb in enumerate(CBO):
                    p_qk = pqk.tile([128, 512], F32, tag="pqk")
                    for kt in range(KT8):
                        nc.tensor.matmul(
                            p_qk, wqkv_sb[:, kt, cb * 128:(cb + 1) * 128],
                            xt_all[:, kt, tsl],
                            start=(kt == 0), stop=(kt == KT8 - 1),
                        )
                    finish_norm()
                    if tb == 3 and i == 3:
                        # last phase-A sqrt just issued: preload the exp
                        # table set now (exp and sqrt live in disjoint ACT
                        # table sets; this hides the 1.28us reload under the
                        # remaining phase-A matmuls)
                        nc.scalar.activation(junk, eps_sb, AF.Exp, scale=1.0)
                    evict_block(p_qk, cb, tb)
                    t = tb * 4 + i
                    ts = slice(t * 128, (t + 1) * 128)
                    p_v = pv.tile([128, V], F32, tag="pv")
                    for kt in range(KT8):
                        nc.tensor.matmul(
                            p_v, xt_all[:, kt, ts], wqkv_sb[:, kt, QK:QK + V],
                            start=(kt == 0), stop=(kt == KT8 - 1),
                        )
                    v_finish(t, p_v)

            # A->B bridge: junk keeps the PE warm while the last evictions
            # free PSUM banks. The two leftover norm chains (tb3's q
            # channels, needed only by the last query chunk ~100us away)
            # get their ssq matmul + eps-eviction here (DVE is idle at the
            # seam); the rstd itself (quake rsqrt + newton, pure SBUF DVE
            # ops) is spread one-op-per-kt across the first phase-B groups
            # so it never touches the ACT exp table or the score ring.
            p_w2 = pqk.tile([128, 512], F32, tag="pqk")
            for _ in range(5):
                nc.tensor.matmul(p_w2[:, 0:256], wjunk[:, 0:128],
                                 wjunk[:, 0:256], start=True, stop=True)
            leftovers = []
            for li in range(2):
                qsb_l, sq_l, cb_l, tb_l = pend.popleft()
                p_ssq = pssq.tile([128, 512], F32, tag="pssq")
                nc.tensor.matmul(p_ssq, inc_sb[:, cb_l // 2, :], sq_l,
                                 start=True, stop=True)
                e_l = singles.tile([128, 512], F32, tag=f"le{li}",
                                   name=f"le{li}")
                nc.vector.tensor_scalar(e_l, p_ssq, 1.0, EPS,
                                        op0=MUL, op1=ADD)
                leftovers.append((e_l, qsb_l, cb_l, tb_l))

        # -------- phase B: attention with the projection interleaved --------
        with ExitStack() as bctx:
            spool = bctx.enter_context(tc.tile_pool(name="ps", bufs=3, space="PSUM"))
            po0p = bctx.enter_context(tc.tile_pool(name="po0", bufs=1, space="PSUM"))
            po1p = bctx.enter_context(tc.tile_pool(name="po1", bufs=1, space="PSUM"))

            def emit_scores(cbp, qsl, kt):
                ps = spool.tile([128, 2, 512], F32, tag="ps")
                ksl = slice(kt * 128, (kt + 1) * 128)
                for hh in range(2):
                    rows = slice(hh * 64, (hh + 1) * 64)
                    nc.tensor.matmul(
                        ps[:, hh, :], kn[rows, cbp, ksl],
                        qn[rows, cbp, qsl], start=True, stop=True,
                    )
                return ps

            def proj_mm(t, pp):
                """partial-projection matmuls for q token tile t into pp"""
                ts = slice(t * 128, (t + 1) * 128)
                for hb in range(2):        # stationary aT block reused over jg
                    for jg in range(2):
                        nc.tensor.matmul(
                            pp[:, jg, :], aT[:, hb, ts],
                            wproj_sb[:, hb, jg * 512:(jg + 1) * 512],
                            start=(hb == 0), stop=(hb == 1),
                        )

            def proj_evict_jg(t, pp, jg, eng):
                ts = slice(t * 128, (t + 1) * 128)
                outsb = outpool.tile([128, 512], BF16, tag="outsb",
                                     name=f"osb{t}{jg}")
                if eng == "scalar":
                    nc.scalar.activation(outsb, pp[:, jg, :], AF.Copy)
                    dma_eng = nc.sync
                else:
                    nc.vector.tensor_copy(outsb, pp[:, jg, :])
                    dma_eng = nc.scalar
                dma_eng.dma_start(
                    out=out_ext[ts, jg * 512:(jg + 1) * 512], in_=outsb
                )

            def proj_evict(t, pp, engs=("scalar", "vector")):
                for jg in range(2):
                    proj_evict_jg(t, pp, jg, engs[jg])

            def exp_tile(ps, kt):
                pt = ptpool.tile([128, 2, 512], BF16, tag="pt")
                if kt in DVE_KTS:
                    nc.vector.tensor_scalar(
                        pt.bitcast(I16), ps, A16, B16, op0=MUL, op1=ADD)
                else:
                    nc.scalar.activation(pt, ps, AF.Exp, scale=0.125)
                return pt

            def leftover_ops(li):
                """closures for one deferred tb3 q-block rstd: quake rsqrt
                seed + one newton step + the qn write, all cheap DVE SBUF
                ops (avoids ACT, whose exp table must stay resident, and
                the score ring). max rel err ~0.18%."""
                e, qsb, cb, tb = leftovers[li]
                tsl = slice(tb * 512, (tb + 1) * 512)
                y0 = singles.tile([128, 512], F32, tag=f"ly{li}",
                                  name=f"ly{li}")
                tn = singles.tile([128, 512], F32, tag=f"lt{li}",
                                  name=f"lt{li}")
                i32 = mybir.dt.int32
                shr = mybir.AluOpType.logical_shift_right
                return [
                    lambda: nc.vector.tensor_scalar(
                        y0.bitcast(i32), e.bitcast(i32), 1, None, op0=shr),
                    lambda: nc.vector.tensor_scalar(
                        y0.bitcast(i32), y0.bitcast(i32), -1, QUAKE,
                        op0=MUL, op1=ADD),
                    lambda: nc.vector.tensor_mul(tn, e, y0),
                    lambda: nc.vector.tensor_mul(tn, tn, y0),
                    lambda: nc.vector.tensor_scalar(tn, tn, -0.5, 1.5,
                                                    op0=MUL, op1=ADD),
                    lambda: nc.vector.tensor_mul(y0, y0, tn),
                    lambda: nc.vector.tensor_tensor(qn[:, cb % 2, tsl],
                                                    qsb, y0, op=MUL),
                ]

            lops = leftover_ops(0) + leftover_ops(1)

            pending = {"pts": [], "projB": None, "ppA": None}

            def run_group(qh4, cbp, nxt):
                qsl = slice(qh4 * 512, (qh4 + 1) * 512)
                # proj tiles of the previous q-chunk, interleaved here
                ptiles = []
                if qh4 > 0:
                    base = (qh4 - 1) * 4
                    ptiles = [base + 2 * cbp, base + 2 * cbp + 1]
                po0 = po0p.tile([128, 512], F32, tag="po0")
                po1 = po1p.tile([128, 512], F32, tag="po1")
                # kts 0,1 may arrive pre-exp'd from the previous group
                pexp = pending["pts"]
                pending["pts"] = []
                if pexp:
                    pss = []
                else:
                    pss = [emit_scores(cbp, qsl, 0), emit_scores(cbp, qsl, 1)]
                prev = None
                for kt in range(NT):
                    if kt < len(pexp):
                        pt = pexp[kt]
                    else:
                        ps = pss.pop(0)
                        pt = exp_tile(ps, kt)
                    if prev is not None:
                        ppt, pkt = prev
                        nc.tensor.matmul(
                            po0, vaug[:, pkt, 2 * cbp, :],
                            ppt[:, 0, :], start=(pkt == 0), stop=False,
                        )
                        nc.tensor.matmul(
                            po1, vaug[:, pkt, 2 * cbp + 1, :],
                            ppt[:, 1, :], start=(pkt == 0), stop=False,
                        )
                    if kt in (0, 2) and pending["projB"] is not None:
                        # boundary proj tile: evict halves on ACT, spaced so
                        # exp(2) slots between them
                        tB, ppB = pending["projB"]
                        proj_evict_jg(tB, ppB, kt // 2, "scalar")
                        if kt == 2:
                            pending["projB"] = None
                    if kt == 7 and ptiles:
                        ppA = spool.tile([128, 2, 512], F32, tag="ps",
                                         name="ppA")
                        pending["ppA"] = ppA
                        proj_mm(ptiles[0], ppA)
                    if kt == 9 and ptiles:
                        proj_evict(ptiles[0], pending["ppA"])
                    if kt + 2 < NT:
                        pss.append(emit_scores(cbp, qsl, kt + 2))
                    elif nxt is not None:
                        # pre-emit the next group's first two score tiles AND
                        # their exps (schraudolph on DVE, which is free here)
                        # so the next group starts with its AV chain unblocked
                        nqsl = slice(nxt[0] * 512, (nxt[0] + 1) * 512)
                        nps = emit_scores(nxt[1], nqsl, kt - 14)
                        npt = ptpool.tile([128, 2, 512], BF16, tag="pt",
                                          name="npt")
                        nc.vector.tensor_scalar(
                            npt.bitcast(I16), nps, A16, B16, op0=MUL, op1=ADD)
                        pending["pts"].append(npt)
                    if kt < 4 and lops:
                        # one deferred-norm DVE op per kt, early in the group
                        # where DVE has slack
                        lops.pop(0)()
                    prev = (pt, kt)
                ppt, pkt = prev
                nc.tensor.matmul(po0, vaug[:, pkt, 2 * cbp, :],
                                 ppt[:, 0, :], start=False, stop=True)
                nc.tensor.matmul(po1, vaug[:, pkt, 2 * cbp + 1, :],
                                 ppt[:, 1, :], start=False, stop=True)
                if ptiles:
                    ppB = spool.tile([128, 2, 512], F32, tag="ps", name="ppB")
                    proj_mm(ptiles[1], ppB)
                    pending["projB"] = (ptiles[1], ppB)
                # normalize, pipelined per head so po0 frees before the
                # next group's first AV: rec0, mul0, rec1, mul1 on DVE.
                # even head: out rows 0:64, denom rows 64:128; odd: swapped
                rec0 = rpool.tile([128, 512], F32, tag="rec")
                nc.vector.reciprocal_approx_fast(rec0, po0)
                nc.vector.tensor_mul(
                    aT[0:64, cbp, qsl], po0[0:64, :], rec0[64:128, :])
                rec1 = rpool.tile([128, 512], F32, tag="rec")
                nc.vector.reciprocal_approx_fast(rec1, po1)
                nc.vector.tensor_mul(
                    aT[64:128, cbp, qsl], po1[64:128, :], rec1[0:64, :])

            groups = [(q, c) for q in range(4) for c in range(2)]
            for gi, (qh4, cbp) in enumerate(groups):
                nxt = groups[gi + 1] if gi + 1 < len(groups) else None
                run_group(qh4, cbp, nxt)

            # ------- tail: last q-chunk's partial projection --------
            # the last boundary proj tile still needs evicting
            tB, ppB = pending["projB"]
            pending["projB"] = None
            proj_evict(tB, ppB)
            # 4 pp tiles up-front (3 spool + the freed po pair); hb0 matmuls
            # first (they only need the older aT half), then hb1, then evicts
            pps = [spool.tile([128, 2, 512], F32, tag="ps", name=f"tpp{i}")
                   for i in range(3)]
            tpo0 = po0p.tile([128, 512], F32, tag="po0")
            tpo1 = po1p.tile([128, 512], F32, tag="po1")
            tiles = (12, 13, 14, 15)

            def tail_dst(i, jg):
                if i < 3:
                    return pps[i][:, jg, :]
                return tpo0 if jg == 0 else tpo1

            for hb in range(2):
                for i, t in enumerate(tiles):
                    ts = slice(t * 128, (t + 1) * 128)
                    for jg in range(2):
                        nc.tensor.matmul(
                            tail_dst(i, jg), aT[:, hb, ts],
                            wproj_sb[:, hb, jg * 512:(jg + 1) * 512],
                            start=(hb == 0), stop=(hb == 1),
                        )
            for i, t in enumerate(tiles):
                ts = slice(t * 128, (t + 1) * 128)
                for jg in range(2):
                    outsb = outpool.tile([128, 512], BF16, tag="outsb",
                                         name=f"tosb{t}{jg}")
                    if jg == 0:
                        nc.scalar.activation(outsb, tail_dst(i, jg), AF.Copy)
                        dma_eng = nc.sync
                    else:
                        nc.vector.tensor_copy(outsb, tail_dst(i, jg))
                        dma_eng = nc.scalar
                    dma_eng.dma_start(
                        out=out_ext[ts, jg * 512:(jg + 1) * 512], in_=outsb
                    )

    nc.finalize()
    return nc


def make_in_maps(x, qkv_w, qkv_b, q_norm_w, k_norm_w, proj_w, proj_b):
    """Shard the full inputs into the 8 per-core input maps."""
    bf = ml_dtypes.bfloat16
    qw = np.tile(q_norm_w.astype(np.float64), HPC)      # [256]
    kw = np.tile(k_norm_w.astype(np.float64), HPC)
    in_maps = []
    for c in range(8):
        b, g = c // 4, c % 4
        ch = np.arange(4 * g * HD, 4 * (g + 1) * HD)    # this core's head channels
        # columns: q (w-folded) | k (w-folded) | v
        wq = qkv_w[:, ch] * qw[None, :]
        wk = qkv_w[:, C + ch] * kw[None, :]
        wv = qkv_w[:, 2 * C + ch]
        wqkv_c = np.concatenate([wq, wk, wv], axis=1)
        bqk = np.concatenate([qkv_b[ch] * qw, qkv_b[C + ch] * kw])  # [512]
        bv = qkv_b[2 * C + ch]
        # block-diag head-incidence with 1/(64 w^2): [p, {q,k}, p']
        inc = np.zeros((128, 2, 128), np.float64)
        blk = (np.arange(128)[:, None] // HD) == (np.arange(128)[None, :] // HD)
        inc[:, 0, :] = blk / (64.0 * np.tile(q_norm_w.astype(np.float64), 2)[:, None] ** 2)
        inc[:, 1, :] = blk / (64.0 * np.tile(k_norm_w.astype(np.float64), 2)[:, None] ** 2)
        # wproj rows for this core as [128 rows of head-pair, pair, C]
        wproj_c = proj_w[ch, :].reshape(2, V // 2, C).transpose(1, 0, 2)
        in_maps.append({
            "x": np.ascontiguousarray(x[b].T).astype(bf),
            "wqkv": np.ascontiguousarray(wqkv_c).astype(bf),
            "bqk": np.ascontiguousarray(bqk.reshape(4, 128).T, np.float32),
            "bv": np.ascontiguousarray(bv, np.float32),
            "inc": np.ascontiguousarray(inc).astype(bf),
            "wproj": np.ascontiguousarray(wproj_c).astype(bf),
        })
    return in_maps


_NC_CACHE = []


def kernel(x, qkv_w, qkv_b, q_norm_w, k_norm_w, proj_w, proj_b,
           _run_kwargs=None, _res_box=None):
    x = np.asarray(x); qkv_w = np.asarray(qkv_w); qkv_b = np.asarray(qkv_b)
    q_norm_w = np.asarray(q_norm_w); k_norm_w = np.asarray(k_norm_w)
    proj_w = np.asarray(proj_w); proj_b = np.asarray(proj_b)

    if not _NC_CACHE:
        _NC_CACHE.append(build_nc())
    nc = _NC_CACHE[0]
    in_maps = make_in_maps(x, qkv_w, qkv_b, q_norm_w, k_norm_w, proj_w, proj_b)
    res = run_bass_kernel_spmd(nc, in_maps, core_ids=list(range(8)),
                               **(_run_kwargs or {}))
    if _res_box is not None:
        _res_box["res"] = res
    out = np.zeros((B, N, C), np.float32)
    for c in range(8):
        out[c // 4] += res.results[c]["out"].astype(np.float32)
    out += proj_b[None, None, :].astype(np.float32)
    return out


if __name__ == "__main__":
    rng = np.random.default_rng(0)
    x = rng.standard_normal((B, N, C)).astype(np.float32)
    qkv_w = (rng.standard_normal((C, 3 * C)) / np.sqrt(C)).astype(np.float32)
    qkv_b = np.zeros((3 * C,), np.float32)
    qn_w = np.ones((HD,), np.float32)
    kn_w = np.ones((HD,), np.float32)
    proj_w = (rng.standard_normal((C, C)) / np.sqrt(C)).astype(np.float32)
    proj_b = np.zeros((C,), np.float32)
    out = kernel(x, qkv_w, qkv_b, qn_w, kn_w, proj_w, proj_b)
    print("out", out.shape, out.dtype, float(np.abs(out).mean()))


# revision 20
# speedup vs baseline: 1.0104x; 1.0104x over previous
"""Fused multi-head attention block on 8 TRN2 NeuronCores.

reference: qkv = x@Wqkv+b; q,k rmsnorm'd per head; softmax(q k^T/sqrt(hd)) v; proj.
Shapes: x [2,2048,1024], H=16 heads, hd=64.

Distribution (no collectives): 8 cores = 2 batches x 4 head-groups (4 heads each).
Core c: batch b=c//4, heads 4g..4g+3 (g=c%4). Each core computes the partial
projection output (proj_w row-sharded over its heads) for its batch; the host
sums the 4 partials per batch and adds proj_b.

Per-core pipeline (bf16 matmul operands, f32 PSUM accumulation):
  A) qkv GEMM with OUTPUT channel-major for q,k (wqkv stationary, x^T moving)
     so q^T/k^T need no PE transposes; v computed token-major (x^T stationary)
     straight into the AV stationary layout. x arrives in 16 fine (kt,half)
     DMA pieces; tb0 runs its 4 qk chains + 2 v chains interleaved per-kt at
     DMA pace so the PE starts ~8us in and never idles long enough to lose
     the HAM p-state. rmsnorm per head: sq = qsb*qsb on DVE (w folded into
     wqkv on host); per-head sum-of-squares via a block-diagonal 1/(64 w^2)
     matmul that REPLICATES the stat across the head's 64 partitions; sqrt on
     ACT; reciprocal_approx_fast on DVE; one bf16 multiply applies rstd.
     k channels are normalized first; the two leftover norm chains at the
     A-tail are q channels needed only by the last query block.
  B) attention per (q-512-chunk, head-pair): the two heads' K=64 score matmuls
     run in separate PE row-groups (partitions 0:64 / 64:128). exp(s/8)
     alternates per k-tile between ACT (Exp activation) and DVE (Schraudolph
     bf16 bit-trick: s*a+b -> int32, read high half-words as bf16). AV
     accumulates per head with stationary [v|ones]/[ones|v] giving out^T and
     the softmax denominator in one pass; po is split per head (1 PSUM bank
     each) so the normalize epilogue (recip + bf16 multiply) pipelines and
     frees po before the next group's first AV needs it. The partial
     projection for the PREVIOUS q-chunk is interleaved into each group (one
     tile mid-group, one at the boundary) so the PE stays dense across group
     boundaries; only the last q-chunk's 4 proj tiles run as a tail.
"""

from collections import deque
from contextlib import ExitStack

import ml_dtypes
import numpy as np

import concourse.bass as bass
import concourse.mybir as mybir
import concourse.tile as tile
from concourse import bacc
from concourse.bass_utils import run_bass_kernel_spmd

B, N, C = 2, 2048, 1024
H, HD = 16, 64
HPC = 4                 # heads per core
NT = N // 128           # 16 k-token tiles
KT8 = C // 128          # 8 contraction tiles for the qkv GEMM
QK = 2 * HPC * HD       # 512 qk channels per core
V = HPC * HD            # 256 v channels per core
EPS = 1e-6
F32 = mybir.dt.float32
BF16 = mybir.dt.bfloat16
I16 = mybir.dt.int16
AF = mybir.ActivationFunctionType
MUL = mybir.AluOpType.mult
ADD = mybir.AluOpType.add

LOG2E = 1.4426950408889634
# exp(s/8) ~= bf16_frombits(int16(s*A16 + B16)): schraudolph with the /8
# softmax scale folded in; B16 centers the sawtooth error (C ~= 5.5/128).
A16 = 128.0 * LOG2E / 8.0
B16 = 16251.0

# which kt tiles' exp goes to DVE (schraudolph) instead of ACT; alternation
# keeps the score ring advancing, and kts 0,1,15 stay on ACT so the DVE
# epilogue (rec+mul) at group boundaries never blocks the exp chain
DVE_KTS = frozenset((2, 4, 6, 8, 10, 12, 14))

CBO = (2, 3, 0, 1)      # k channel-blocks first: A-tail leftovers are q-only

QUAKE = 0x5F3759DF      # rsqrt seed magic (gpsimd leftover-norm path)


def build_nc():
    nc = bacc.Bacc("TRN2", target_bir_lowering=False, debug=False)

    x_ext = nc.declare_dram_parameter("x", [C, N], BF16, isOutput=False)
    wqkv_ext = nc.declare_dram_parameter("wqkv", [C, QK + V], BF16, isOutput=False)
    bqk_ext = nc.declare_dram_parameter("bqk", [128, 4], F32, isOutput=False)
    bv_ext = nc.declare_dram_parameter("bv", [V], F32, isOutput=False)
    inc_ext = nc.declare_dram_parameter("inc", [128, 2, 128], BF16, isOutput=False)
    wproj_ext = nc.declare_dram_parameter("wproj", [V // 2, 2, C], BF16, isOutput=False)
    # bf16 partials: halves the 8 MB output drain; host sums in f32
    out_ext = nc.declare_dram_parameter("out", [N, C], BF16, isOutput=True)

    with tile.TileContext(nc) as tc, ExitStack() as ctx:
        singles = ctx.enter_context(tc.tile_pool(name="singles", bufs=1))

        wqkv_sb = singles.tile([128, KT8, QK + V], BF16, tag="wqkv")
        xt_all = singles.tile([128, KT8, N], BF16, tag="xt_all")
        # x in 16 fine pieces: h0 (tokens 0:1024) kt-major on the scalar
        # queue so (w[kt], x[kt,h0]) pairs land together ~1.25us apart and
        # tb0/tb1 can consume them at DMA pace; w + h1 on the sync queue.
        for kt in range(KT8):
            nc.scalar.dma_start(
                out=xt_all[:, kt, 0:1024],
                in_=x_ext[kt * 128:(kt + 1) * 128, 0:1024])
        for kt in range(KT8):
            nc.sync.dma_start(
                out=wqkv_sb[:, kt, :], in_=wqkv_ext[kt * 128:(kt + 1) * 128, :]
            )
        bqk_sb = singles.tile([128, 4], F32, tag="bqk")
        nc.sync.dma_start(out=bqk_sb, in_=bqk_ext[:, :])
        bv_sb = singles.tile([128, V], F32, tag="bv")
        nc.sync.dma_start(out=bv_sb, in_=bv_ext[:].partition_broadcast(128))
        inc_sb = singles.tile([128, 2, 128], BF16, tag="inc")
        nc.sync.dma_start(out=inc_sb, in_=inc_ext[:, :, :])
        for kt in range(KT8):
            nc.sync.dma_start(
                out=xt_all[:, kt, 1024:2048],
                in_=x_ext[kt * 128:(kt + 1) * 128, 1024:2048])
        wproj_sb = singles.tile([128, 2, C], BF16, tag="wproj")
        nc.sync.dma_start(out=wproj_sb, in_=wproj_ext[:, :, :])
        eps_sb = singles.tile([128, 1], F32, tag="eps")
        nc.vector.memset(eps_sb, EPS)
        junk = singles.tile([128, 1], F32, tag="junk")

        # persistent activations (all channel-major)
        qn = singles.tile([128, 2, N], BF16, tag="qn")
        kn = singles.tile([128, 2, N], BF16, tag="kn")
        # AV stationary blocks: even local head: [v | ones]; odd: [ones | v]
        # memset on gpsimd: it is otherwise idle and finishes by ~14us,
        # before the first v_finish writes (~18us)
        vaug = singles.tile([128, NT, HPC, 128], BF16, tag="vaug")
        nc.gpsimd.memset(vaug, 1.0)
        # normalized attention output; block hb packs heads 2hb,2hb+1
        aT = singles.tile([128, 2, N], BF16, tag="aT")

        # phase-B SBUF pools allocated BEFORE phase A's so that B's first
        # tiles never alias just-freed A tiles (the aliasing serialized the
        # first exp behind the A-tail norm chains)
        ptpool = ctx.enter_context(tc.tile_pool(name="pt", bufs=4))
        rpool = ctx.enter_context(tc.tile_pool(name="rec", bufs=4))
        outpool = ctx.enter_context(tc.tile_pool(name="outsb", bufs=6))

        # phase-A SBUF pools stay open through phase B: the two leftover
        # norm chains (tb3's q channels, needed only by the last q-chunk)
        # are finished inside phase B from these tiles
        qsbp = ctx.enter_context(tc.tile_pool(name="qsb", bufs=3))
        sqp = ctx.enter_context(tc.tile_pool(name="sqp", bufs=2))
        smp = ctx.enter_context(tc.tile_pool(name="smp", bufs=2))
        rstdp = ctx.enter_context(tc.tile_pool(name="rstd", bufs=2))

        # ---------------- phase A: qkv + rmsnorm, no transposes --------------
        with ExitStack() as actx:
            pqk = actx.enter_context(tc.tile_pool(name="pqk", bufs=4, space="PSUM"))
            pv = actx.enter_context(tc.tile_pool(name="pv", bufs=2, space="PSUM"))
            pssq = actx.enter_context(tc.tile_pool(name="pssq", bufs=2, space="PSUM"))

            # short PE warmup during the DMA lead-in (first data ~7.9us)
            wjunk = qsbp.tile([128, 512], BF16, tag="qsb")
            nc.vector.memset(wjunk, 0.0)
            p_warm = pssq.tile([128, 512], F32, tag="pssq")
            for _ in range(6):
                nc.tensor.matmul(p_warm[:, 0:128], wjunk[:, 0:128],
                                 wjunk[:, 0:128], start=True, stop=True)

            pend = deque()  # (qsb, sq, cb, tb) awaiting ssq matmul + rstd chain

            def evict_block(p_qk, cb, tb):
                """f32 PSUM -> bf16 qsb (+bias) on ACT; sq = qsb^2 on DVE."""
                qsb = qsbp.tile([128, 512], BF16, tag="qsb")
                nc.scalar.activation(qsb, p_qk, AF.Identity,
                                     bias=bqk_sb[:, cb:cb + 1], scale=1.0)
                sq = sqp.tile([128, 512], BF16, tag="sq")
                nc.vector.tensor_mul(sq, qsb, qsb)
                pend.append((qsb, sq, cb, tb))

            def finish_norm():
                qsb, sq, cb, tb = pend.popleft()
                tsl = slice(tb * 512, (tb + 1) * 512)
                p_ssq = pssq.tile([128, 512], F32, tag="pssq")
                nc.tensor.matmul(p_ssq, inc_sb[:, cb // 2, :], sq,
                                 start=True, stop=True)
                sm = smp.tile([128, 512], F32, tag="sm")
                nc.scalar.activation(sm, p_ssq, AF.Sqrt,
                                     bias=eps_sb[:, 0:1], scale=1.0)
                rstd = rstdp.tile([128, 512], F32, tag="rstd")
                nc.vector.reciprocal_approx_fast(rstd, sm)
                dst = qn if cb < 2 else kn
                nc.vector.tensor_tensor(dst[:, cb % 2, tsl], qsb, rstd, op=MUL)

            def v_finish(t, p_v):
                pv3 = p_v.rearrange("p (h d) -> p h d", d=HD)
                bv3 = bv_sb.rearrange("p (h d) -> p h d", d=HD)
                nc.vector.tensor_add(vaug[:, t, 0::2, 0:HD], pv3[:, 0::2, :],
                                     bv3[:, 0::2, :])
                nc.vector.tensor_add(vaug[:, t, 1::2, HD:128], pv3[:, 1::2, :],
                                     bv3[:, 1::2, :])

            # --- tb0: 4 qk chains + 2 v chains interleaved per-kt, paced by
            # the (w[kt], x[kt,h0]) DMA arrivals; junk matmuls pad the seams
            tsl0 = slice(0, 512)
            chains = {cb: pqk.tile([128, 512], F32, tag="pqk", name=f"chain{cb}")
                      for cb in CBO}
            pv01 = [pv.tile([128, V], F32, tag="pv", name=f"pv0{t}")
                    for t in range(2)]
            for kt in range(KT8):
                for cb in CBO:
                    nc.tensor.matmul(
                        chains[cb], wqkv_sb[:, kt, cb * 128:(cb + 1) * 128],
                        xt_all[:, kt, tsl0],
                        start=(kt == 0), stop=(kt == KT8 - 1),
                    )
                for t in range(2):
                    ts = slice(t * 128, (t + 1) * 128)
                    nc.tensor.matmul(
                        pv01[t], xt_all[:, kt, ts], wqkv_sb[:, kt, QK:QK + V],
                        start=(kt == 0), stop=(kt == KT8 - 1),
                    )
                if kt < KT8 - 1:
                    for _ in range(2):
                        nc.tensor.matmul(p_warm[:, 0:128], wjunk[:, 0:128],
                                         wjunk[:, 0:128], start=True, stop=True)
            for cb in CBO:
                evict_block(chains[cb], cb, 0)
            for t in range(2):
                v_finish(t, pv01[t])
            # bridge: v t2,t3 chains + first two norm finishes cover the ACT
            # eviction latency before tb1's chains reuse the pqk ring
            for t in (2, 3):
                p_v = pv.tile([128, V], F32, tag="pv")
                ts = slice(t * 128, (t + 1) * 128)
                for kt in range(KT8):
                    nc.tensor.matmul(
                        p_v, xt_all[:, kt, ts], wqkv_sb[:, kt, QK:QK + V],
                        start=(kt == 0), stop=(kt == KT8 - 1),
                    )
                finish_norm()
                v_finish(t, p_v)

            # --- tb1..tb3: sequential blocks, one v tile + one norm finish
            # per block (pend stays 2 deep; leftovers are tb3's q channels)
            for tb in range(1, 4):
                tsl = slice(tb * 512, (tb + 1) * 512)
                for i, cb in enumerate(CBO):
                    p_qk = pqk.tile([128, 512], F32, tag="pqk")
                    for kt in range(KT8):
                        nc.tensor.matmul(
                            p_qk, wqkv_sb[:, kt, cb * 128:(cb + 1) * 128],
                            xt_all[:, kt, tsl],
                            start=(kt == 0), stop=(kt == KT8 - 1),
                        )
                    finish_norm()
                    if tb == 3 and i == 3:
                        # last phase-A sqrt just issued: preload the exp
                        # table set now (exp and sqrt live in disjoint ACT
                        # table sets; this hides the 1.28us reload under the
                        # remaining phase-A matmuls)
                        nc.scalar.activation(junk, eps_sb, AF.Exp, scale=1.0)
                    evict_block(p_qk, cb, tb)
                    t = tb * 4 + i
                    ts = slice(t * 128, (t + 1) * 128)
                    p_v = pv.tile([128, V], F32, tag="pv")
                    for kt in range(KT8):
                        nc.tensor.matmul(
                            p_v, xt_all[:, kt, ts], wqkv_sb[:, kt, QK:QK + V],
                            start=(kt == 0), stop=(kt == KT8 - 1),
                        )
                    v_finish(t, p_v)

            # A->B bridge: junk keeps the PE warm while the last evictions
            # free PSUM banks. The two leftover norm chains (tb3's q
            # channels, needed only by the last query chunk ~100us away)
            # get their ssq matmul + eps-eviction here (DVE is idle at the
            # seam); the rstd itself (quake rsqrt + newton, pure SBUF DVE
            # ops) is spread one-op-per-kt across the first phase-B groups
            # so it never touches the ACT exp table or the score ring.
            p_w2 = pqk.tile([128, 512], F32, tag="pqk")
            for _ in range(5):
                nc.tensor.matmul(p_w2[:, 0:256], wjunk[:, 0:128],
                                 wjunk[:, 0:256], start=True, stop=True)
            leftovers = []
            for li in range(2):
                qsb_l, sq_l, cb_l, tb_l = pend.popleft()
                p_ssq = pssq.tile([128, 512], F32, tag="pssq")
                nc.tensor.matmul(p_ssq, inc_sb[:, cb_l // 2, :], sq_l,
                                 start=True, stop=True)
                e_l = singles.tile([128, 512], F32, tag=f"le{li}",
                                   name=f"le{li}")
                nc.vector.tensor_scalar(e_l, p_ssq, 1.0, EPS,
                                        op0=MUL, op1=ADD)
                leftovers.append((e_l, qsb_l, cb_l, tb_l))

        # -------- phase B: attention with the projection interleaved --------
        with ExitStack() as bctx:
            spool = bctx.enter_context(tc.tile_pool(name="ps", bufs=2, space="PSUM"))
            po0p = bctx.enter_context(tc.tile_pool(name="po0", bufs=1, space="PSUM"))
            po1p = bctx.enter_context(tc.tile_pool(name="po1", bufs=1, space="PSUM"))
            ppp = bctx.enter_context(tc.tile_pool(name="ppp", bufs=1, space="PSUM"))

            def emit_scores(cbp, qsl, kt):
                ps = spool.tile([128, 2, 512], F32, tag="ps")
                ksl = slice(kt * 128, (kt + 1) * 128)
                for hh in range(2):
                    rows = slice(hh * 64, (hh + 1) * 64)
                    nc.tensor.matmul(
                        ps[:, hh, :], kn[rows, cbp, ksl],
                        qn[rows, cbp, qsl], start=True, stop=True,
                    )
                return ps

            def proj_mm(t, pp):
                """partial-projection matmuls for q token tile t into pp"""
                ts = slice(t * 128, (t + 1) * 128)
                for hb in range(2):        # stationary aT block reused over jg
                    for jg in range(2):
                        nc.tensor.matmul(
                            pp[:, jg, :], aT[:, hb, ts],
                            wproj_sb[:, hb, jg * 512:(jg + 1) * 512],
                            start=(hb == 0), stop=(hb == 1),
                        )

            def proj_evict_jg(t, pp, jg, eng):
                ts = slice(t * 128, (t + 1) * 128)
                outsb = outpool.tile([128, 512], BF16, tag="outsb",
                                     name=f"osb{t}{jg}")
                if eng == "scalar":
                    nc.scalar.activation(outsb, pp[:, jg, :], AF.Copy)
                    dma_eng = nc.sync
                else:
                    nc.vector.tensor_copy(outsb, pp[:, jg, :])
                    dma_eng = nc.scalar
                dma_eng.dma_start(
                    out=out_ext[ts, jg * 512:(jg + 1) * 512], in_=outsb
                )

            def proj_evict(t, pp, engs=("scalar", "vector")):
                for jg in range(2):
                    proj_evict_jg(t, pp, jg, engs[jg])

            def exp_tile(ps, kt):
                pt = ptpool.tile([128, 2, 512], BF16, tag="pt")
                if kt in DVE_KTS:
                    nc.vector.tensor_scalar(
                        pt.bitcast(I16), ps, A16, B16, op0=MUL, op1=ADD)
                else:
                    nc.scalar.activation(pt, ps, AF.Exp, scale=0.125)
                return pt

            def leftover_ops(li):
                """closures for one deferred tb3 q-block rstd: quake rsqrt
                seed + one newton step + the qn write, all cheap DVE SBUF
                ops (avoids ACT, whose exp table must stay resident, and
                the score ring). max rel err ~0.18%."""
                e, qsb, cb, tb = leftovers[li]
                tsl = slice(tb * 512, (tb + 1) * 512)
                y0 = singles.tile([128, 512], F32, tag=f"ly{li}",
                                  name=f"ly{li}")
                tn = singles.tile([128, 512], F32, tag=f"lt{li}",
                                  name=f"lt{li}")
                i32 = mybir.dt.int32
                shr = mybir.AluOpType.logical_shift_right
                return [
                    lambda: nc.vector.tensor_scalar(
                        y0.bitcast(i32), e.bitcast(i32), 1, None, op0=shr),
                    lambda: nc.vector.tensor_scalar(
                        y0.bitcast(i32), y0.bitcast(i32), -1, QUAKE,
                        op0=MUL, op1=ADD),
                    lambda: nc.vector.tensor_mul(tn, e, y0),
                    lambda: nc.vector.tensor_mul(tn, tn, y0),
                    lambda: nc.vector.tensor_scalar(tn, tn, -0.5, 1.5,
                                                    op0=MUL, op1=ADD),
                    lambda: nc.vector.tensor_mul(y0, y0, tn),
                    lambda: nc.vector.tensor_tensor(qn[:, cb % 2, tsl],
                                                    qsb, y0, op=MUL),
                ]

            lops = leftover_ops(0) + leftover_ops(1)

            pending = {"ppA": None}

            def run_group(qh4, cbp, nxt):
                qsl = slice(qh4 * 512, (qh4 + 1) * 512)
                # proj tiles of the previous q-chunk, interleaved here
                ptiles = []
                if qh4 > 0:
                    base = (qh4 - 1) * 4
                    ptiles = [base + 2 * cbp, base + 2 * cbp + 1]
                po0 = po0p.tile([128, 512], F32, tag="po0")
                po1 = po1p.tile([128, 512], F32, tag="po1")
                pss = [emit_scores(cbp, qsl, 0), emit_scores(cbp, qsl, 1)]
                prev = None
                for kt in range(NT):
                    ps = pss.pop(0)
                    pt = exp_tile(ps, kt)
                    if prev is not None:
                        ppt, pkt = prev
                        nc.tensor.matmul(
                            po0, vaug[:, pkt, 2 * cbp, :],
                            ppt[:, 0, :], start=(pkt == 0), stop=False,
                        )
                        nc.tensor.matmul(
                            po1, vaug[:, pkt, 2 * cbp + 1, :],
                            ppt[:, 1, :], start=(pkt == 0), stop=False,
                        )
                    if kt == 7 and ptiles:
                        ppA = ppp.tile([128, 2, 512], F32, tag="pp",
                                       name="ppA")
                        pending["ppA"] = ppA
                        proj_mm(ptiles[0], ppA)
                    if kt == 9 and ptiles:
                        proj_evict(ptiles[0], pending["ppA"])
                    if kt + 2 < NT:
                        pss.append(emit_scores(cbp, qsl, kt + 2))
                    if kt < 4 and lops:
                        # one deferred-norm DVE op per kt, early in the group
                        # where DVE has slack
                        lops.pop(0)()
                    prev = (pt, kt)
                ppt, pkt = prev
                nc.tensor.matmul(po0, vaug[:, pkt, 2 * cbp, :],
                                 ppt[:, 0, :], start=False, stop=True)
                nc.tensor.matmul(po1, vaug[:, pkt, 2 * cbp + 1, :],
                                 ppt[:, 1, :], start=False, stop=True)
                if ptiles:
                    ppB = ppp.tile([128, 2, 512], F32, tag="pp", name="ppB")
                    proj_mm(ptiles[1], ppB)
                    proj_evict(ptiles[1], ppB)
                # normalize, pipelined per head so po0 frees before the
                # next group's first AV: rec0, mul0, rec1, mul1 on DVE.
                # even head: out rows 0:64, denom rows 64:128; odd: swapped
                rec0 = rpool.tile([128, 512], F32, tag="rec")
                nc.vector.reciprocal_approx_fast(rec0, po0)
                nc.vector.tensor_mul(
                    aT[0:64, cbp, qsl], po0[0:64, :], rec0[64:128, :])
                rec1 = rpool.tile([128, 512], F32, tag="rec")
                nc.vector.reciprocal_approx_fast(rec1, po1)
                nc.vector.tensor_mul(
                    aT[64:128, cbp, qsl], po1[64:128, :], rec1[0:64, :])

            groups = [(q, c) for q in range(4) for c in range(2)]
            for gi, (qh4, cbp) in enumerate(groups):
                nxt = groups[gi + 1] if gi + 1 < len(groups) else None
                run_group(qh4, cbp, nxt)

            # ------- tail: last q-chunk's partial projection --------
            # 4 pp tiles up-front (ppp + 2 spool + the freed po pair); all
            # hb0 matmuls first (they only need the older aT half), then per
            # tile: hb1 matmuls followed by its evicts so the out DMAs
            # overlap the remaining matmuls. All DMA issues go to the sync
            # queue: the scalar queue shares the ACT engine with the copies.
            pp12 = ppp.tile([128, 2, 512], F32, tag="pp", name="pp12")
            pps = [pp12,
                   spool.tile([128, 2, 512], F32, tag="ps", name="tpp1"),
                   spool.tile([128, 2, 512], F32, tag="ps", name="tpp2")]
            tpo0 = po0p.tile([128, 512], F32, tag="po0")
            tpo1 = po1p.tile([128, 512], F32, tag="po1")
            tiles = (12, 13, 14, 15)

            def tail_dst(i, jg):
                if i < 3:
                    return pps[i][:, jg, :]
                return tpo0 if jg == 0 else tpo1

            for i, t in enumerate(tiles):
                ts = slice(t * 128, (t + 1) * 128)
                for jg in range(2):
                    nc.tensor.matmul(
                        tail_dst(i, jg), aT[:, 0, ts],
                        wproj_sb[:, 0, jg * 512:(jg + 1) * 512],
                        start=True, stop=False,
                    )
            for i, t in enumerate(tiles):
                ts = slice(t * 128, (t + 1) * 128)
                for jg in range(2):
                    nc.tensor.matmul(
                        tail_dst(i, jg), aT[:, 1, ts],
                        wproj_sb[:, 1, jg * 512:(jg + 1) * 512],
                        start=False, stop=True,
                    )
                for jg in range(2):
                    outsb = outpool.tile([128, 512], BF16, tag="outsb",
                                         name=f"tosb{t}{jg}")
                    if jg == 0:
                        nc.scalar.activation(outsb, tail_dst(i, jg), AF.Copy)
                    else:
                        nc.vector.tensor_copy(outsb, tail_dst(i, jg))
                    nc.sync.dma_start(
                        out=out_ext[ts, jg * 512:(jg + 1) * 512], in_=outsb
                    )

    nc.finalize()
    return nc


def make_in_maps(x, qkv_w, qkv_b, q_norm_w, k_norm_w, proj_w, proj_b):
    """Shard the full inputs into the 8 per-core input maps."""
    bf = ml_dtypes.bfloat16
    qw = np.tile(q_norm_w.astype(np.float64), HPC)      # [256]
    kw = np.tile(k_norm_w.astype(np.float64), HPC)
    in_maps = []
    for c in range(8):
        b, g = c // 4, c % 4
        ch = np.arange(4 * g * HD, 4 * (g + 1) * HD)    # this core's head channels
        # columns: q (w-folded) | k (w-folded) | v
        wq = qkv_w[:, ch] * qw[None, :]
        wk = qkv_w[:, C + ch] * kw[None, :]
        wv = qkv_w[:, 2 * C + ch]
        wqkv_c = np.concatenate([wq, wk, wv], axis=1)
        bqk = np.concatenate([qkv_b[ch] * qw, qkv_b[C + ch] * kw])  # [512]
        bv = qkv_b[2 * C + ch]
        # block-diag head-incidence with 1/(64 w^2): [p, {q,k}, p']
        inc = np.zeros((128, 2, 128), np.float64)
        blk = (np.arange(128)[:, None] // HD) == (np.arange(128)[None, :] // HD)
        inc[:, 0, :] = blk / (64.0 * np.tile(q_norm_w.astype(np.float64), 2)[:, None] ** 2)
        inc[:, 1, :] = blk / (64.0 * np.tile(k_norm_w.astype(np.float64), 2)[:, None] ** 2)
        # wproj rows for this core as [128 rows of head-pair, pair, C]
        wproj_c = proj_w[ch, :].reshape(2, V // 2, C).transpose(1, 0, 2)
        in_maps.append({
            "x": np.ascontiguousarray(x[b].T).astype(bf),
            "wqkv": np.ascontiguousarray(wqkv_c).astype(bf),
            "bqk": np.ascontiguousarray(bqk.reshape(4, 128).T, np.float32),
            "bv": np.ascontiguousarray(bv, np.float32),
            "inc": np.ascontiguousarray(inc).astype(bf),
            "wproj": np.ascontiguousarray(wproj_c).astype(bf),
        })
    return in_maps


_NC_CACHE = []


def kernel(x, qkv_w, qkv_b, q_norm_w, k_norm_w, proj_w, proj_b,
           _run_kwargs=None, _res_box=None):
    x = np.asarray(x); qkv_w = np.asarray(qkv_w); qkv_b = np.asarray(qkv_b)
    q_norm_w = np.asarray(q_norm_w); k_norm_w = np.asarray(k_norm_w)
    proj_w = np.asarray(proj_w); proj_b = np.asarray(proj_b)

    if not _NC_CACHE:
        _NC_CACHE.append(build_nc())
    nc = _NC_CACHE[0]
    in_maps = make_in_maps(x, qkv_w, qkv_b, q_norm_w, k_norm_w, proj_w, proj_b)
    res = run_bass_kernel_spmd(nc, in_maps, core_ids=list(range(8)),
                               **(_run_kwargs or {}))
    if _res_box is not None:
        _res_box["res"] = res
    out = np.zeros((B, N, C), np.float32)
    for c in range(8):
        out[c // 4] += res.results[c]["out"].astype(np.float32)
    out += proj_b[None, None, :].astype(np.float32)
    return out


if __name__ == "__main__":
    rng = np.random.default_rng(0)
    x = rng.standard_normal((B, N, C)).astype(np.float32)
    qkv_w = (rng.standard_normal((C, 3 * C)) / np.sqrt(C)).astype(np.float32)
    qkv_b = np.zeros((3 * C,), np.float32)
    qn_w = np.ones((HD,), np.float32)
    kn_w = np.ones((HD,), np.float32)
    proj_w = (rng.standard_normal((C, C)) / np.sqrt(C)).astype(np.float32)
    proj_b = np.zeros((C,), np.float32)
    out = kernel(x, qkv_w, qkv_b, qn_w, kn_w, proj_w, proj_b)
    print("out", out.shape, out.dtype, float(np.abs(out).mean()))
